# revision 1
# baseline (speedup 1.0000x reference)
"""Multi-head causal self-attention on 8 Trainium2 NeuronCores.

Problem: B=4, S=2048, D=1024, H=16 heads (dk=64), fp32, causal softmax.

Sharding: hybrid batch x head-group. Core c handles batch b = c//2 and head
group g = c%2 (8 heads = 512 dims). Each core computes QKV projections for
its head group, causal flash-style attention in scores-transposed layout,
and a partial output projection over its 512 context dims. The host sums
the two partials per batch.

Device-side layout choices (per core):
  - All matmuls run in float32r (TF32-like, ~1e-4 rel err, full PE speed).
  - Q^T, K^T stored [dk-major]: tile [128, 4, 2048]; partition block of 128
    = one head PAIR (64 rows head 2j, 64 rows head 2j+1) so score matmuls
    (contraction dk=64) row-pack two heads concurrently in the PE array.
  - Scores computed transposed: S^T[k, q] tiles [128 k, 512 q], exp on the
    scalar engine, causal handling by narrowing matmuls to q >= k-chunk
    start plus one triangular 128x128 mask multiply per diagonal block.
  - V stored [k-part, dv] with a ones-column appended (65 wide): the P@V
    matmul (lhsT=V_aug, rhs=exp(S^T)) then yields both the context and the
    softmax denominator (row 64) in one pass, accumulated over k chunks in
    PSUM. Normalization is deferred to after P@V.
  - Denominator reciprocal is broadcast across the 128 partitions of a head
    pair with a tiny [2,128] ones-pattern matmul, then applied with one
    vector multiply per context tile.
"""

import numpy as np
from contextlib import ExitStack

import concourse.bass as bass
import concourse.tile as tile
from concourse import bacc, mybir
from concourse.bass_utils import run_bass_kernel_spmd

B, S, D = 4, 2048, 1024
H = 16
DK = 64
G = 2              # head groups (cores per batch)
HD = D // G        # per-core head dims = 512 (8 heads)
NH = HD // DK      # heads per core = 8
P = 128
NPAIR = NH // 2    # head pairs per core = 4
QC = 512           # q free-dim chunk
NQC = S // QC      # 4
NKC = S // P       # 16 k chunks
KO = D // P        # 8 contraction chunks for projections

F32 = mybir.dt.float32
F32R = mybir.dt.float32r
EXP = mybir.ActivationFunctionType.Exp

_CACHE: dict = {}


def _emit(ctx: ExitStack, tc, xt, wq, wk, wv, wo, tri, m2, ones, out):
    nc = tc.nc

    # ---- persistent SBUF tensors -------------------------------------
    persist = ctx.enter_context(tc.tile_pool(name="persist", bufs=1))
    qt_sb = persist.tile([P, NPAIR, S], F32R)   # Q^T/8, pair-major
    kt_sb = persist.tile([P, NPAIR, S], F32R)   # K^T
    v_sb = persist.tile([P, NKC, NH, DK + 1], F32R)  # V + ones col
    tri_sb = persist.tile([P, P], F32R)
    m2_sb = persist.tile([NH, NPAIR * P], F32R)

    # ---- phase 1: Q/K projections + first V window --------------------
    # V for k-chunks 0..3 is computed here (attention needs it first);
    # the remaining V windows are computed lazily inside the attention
    # loop, where the scalar-engine-bound exp leaves PE slack.
    xt_r = xt.rearrange("(o p) s -> p o s", p=P)
    with (
        tc.tile_pool(name="xpool", bufs=1) as xpool,
        tc.tile_pool(name="wpool", bufs=2) as wpool,
        tc.tile_pool(name="pjps", bufs=4, space="PSUM") as pjps,
    ):
        x_sb = xpool.tile([P, KO, S], F32R)
        # weights first: they unblock the first matmuls.  wq/wk/wv share two
        # 16KB slots (tag "w"): wv's load starts as soon as the Q pass ends.
        wq_sb = wpool.tile([P, KO, HD], F32R, tag="w")
        wq_r = wq.rearrange("(o p) m -> p o m", p=P)
        nc.scalar.dma_start(wq_sb[:, :, 0:HD // 2], wq_r[:, :, 0:HD // 2])
        nc.gpsimd.dma_start(wq_sb[:, :, HD // 2:], wq_r[:, :, HD // 2:])
        wk_sb = wpool.tile([P, KO, HD], F32R, tag="w")
        # spread the big activation load over the three DMA-capable queues
        x_engs = [nc.sync, nc.gpsimd, nc.scalar]
        for ko in range(KO):
            x_engs[ko % 3].dma_start(x_sb[:, ko, :], xt_r[:, ko, :])
        # wk and the constants are not needed until later; load behind x
        nc.sync.dma_start(wk_sb[:], wk.rearrange("(o p) m -> p o m", p=P))
        nc.gpsimd.dma_start(tri_sb[:], tri)
        nc.gpsimd.dma_start(m2_sb[:], m2)
        ones_sb = wpool.tile([P, NKC * NH], F32R, tag="ones")
        nc.gpsimd.dma_start(ones_sb[:], ones)
        nc.vector.tensor_copy(
            v_sb[:, :, :, DK],
            ones_sb.rearrange("p (a b) -> p a b", a=NKC))

        # first four Q groups run ko-outer-interleaved across the four
        # PSUM slots, so the PE advances on every x chunk as it arrives
        # instead of stalling inside one accumulation group
        first_q = [pjps.tile([P, QC], F32, tag="pj", name=f"qps{sc}")
                   for sc in range(NQC)]
        for ko in range(KO):
            for sc in range(NQC):
                nc.tensor.matmul(
                    first_q[sc][:],
                    wq_sb[:, ko, 0:P],
                    x_sb[:, ko, sc * QC:(sc + 1) * QC],
                    start=(ko == 0), stop=(ko == KO - 1),
                )
        for sc in range(NQC):
            nc.scalar.mul(
                qt_sb[:, 0, sc * QC:(sc + 1) * QC], first_q[sc][:], 1.0 / 8.0)

        for m in range(1, NPAIR):
            for sc in range(NQC):
                qps = pjps.tile([P, QC], F32, tag="pj")
                for ko in range(KO):
                    nc.tensor.matmul(
                        qps[:],
                        wq_sb[:, ko, m * P:(m + 1) * P],
                        x_sb[:, ko, sc * QC:(sc + 1) * QC],
                        start=(ko == 0), stop=(ko == KO - 1),
                    )
                # fold in the 1/sqrt(dk) softmax scale here
                # (scalar engine is idle during projections)
                nc.scalar.mul(
                    qt_sb[:, m, sc * QC:(sc + 1) * QC], qps[:], 1.0 / 8.0)

        wv_sb = wpool.tile([P, KO, HD], F32R, tag="w")
        nc.sync.dma_start(wv_sb[:], wv.rearrange("(o p) m -> p o m", p=P))

        for m in range(NPAIR):
            for sc in range(NQC):
                kps = pjps.tile([P, QC], F32, tag="pj")
                for ko in range(KO):
                    nc.tensor.matmul(
                        kps[:],
                        wk_sb[:, ko, m * P:(m + 1) * P],
                        x_sb[:, ko, sc * QC:(sc + 1) * QC],
                        start=(ko == 0), stop=(ko == KO - 1),
                    )
                nc.vector.tensor_copy(
                    kt_sb[:, m, sc * QC:(sc + 1) * QC], kps[:])

        for sc in range(4):
            vps = pjps.tile([P, HD], F32, tag="pj")
            for ko in range(KO):
                nc.tensor.matmul(
                    vps[:],
                    x_sb[:, ko, sc * P:(sc + 1) * P],
                    wv_sb[:, ko, :],
                    start=(ko == 0), stop=(ko == KO - 1),
                )
            nc.vector.tensor_copy(
                v_sb[:, sc, :, 0:DK],
                vps.rearrange("p (h e) -> p h e", h=NH))

    # ---- phase 2: attention + lazy V + normalization + out proj ------
    persist2 = ctx.enter_context(tc.tile_pool(name="persist2", bufs=1))
    ctx_sb = persist2.tile([P, NPAIR, NQC, QC], F32R)  # context^T, pair-major

    # q-chunk outer so the (PE-light, ACT-heavy) attention of chunk qc
    # overlaps the (PE-heavy) output projection of chunk qc-1 and the
    # V projection for the next k-window.
    with (
        tc.tile_pool(name="wop", bufs=1) as wop,
        tc.tile_pool(name="spps", bufs=2, space="PSUM") as spps,
        tc.tile_pool(name="otps", bufs=1, space="PSUM") as otps,
        tc.tile_pool(name="mixps", bufs=2, space="PSUM") as mixps,
        tc.tile_pool(name="ptpool", bufs=3) as ptpool,
        tc.tile_pool(name="bnpool", bufs=2) as bnpool,
        tc.tile_pool(name="osb", bufs=3) as osb,
        tc.tile_pool(name="srpool", bufs=2) as srpool,
        tc.tile_pool(name="xvpool", bufs=2) as xvpool,
        tc.tile_pool(name="wvp2", bufs=1) as wvp2,
    ):
        wo_sb = wop.tile([P, NPAIR, D], F32R)
        nc.sync.dma_start(wo_sb[:], wo.rearrange("(j p) o -> p j o", p=P))
        wv2_sb = wvp2.tile([P, KO, HD], F32R)
        nc.sync.dma_start(wv2_sb[:], wv.rearrange("(o p) m -> p o m", p=P))

        def emit_norm_oproj(qc, rcp_t, j):
            """Normalization + output projection for pair j of chunk qc."""
            rp = mixps.tile([P, QC], F32, tag="mix", name="rp")
            nc.tensor.matmul(
                rp[:], m2_sb[0:NH, j * P:(j + 1) * P],
                rcp_t[:], start=True, stop=True)
            nc.vector.tensor_mul(
                ctx_sb[:, j, qc, :], ctx_sb[:, j, qc, :], rp[:])

        def emit_oproj_group(qc, t, no):
            op = mixps.tile([P, QC], F32, tag="mix", name="op")
            for j in range(NPAIR):
                nc.tensor.matmul(
                    op[:],
                    ctx_sb[:, j, qc, t * P:(t + 1) * P],
                    wo_sb[:, j, no * QC:(no + 1) * QC],
                    start=(j == 0), stop=(j == NPAIR - 1),
                )
            o_sb = osb.tile([P, QC], F32, tag="o_sb", name="o_sb")
            nc.vector.tensor_copy(o_sb[:], op[:])
            sc = qc * (QC // P) + t
            nc.sync.dma_start(
                out[sc * P:(sc + 1) * P, no * QC:(no + 1) * QC], o_sb[:])

        pending = None  # (qc, rcp_t) whose norm+O-proj is deferred
        for qc in range(NQC):
            qcs = slice(qc * QC, (qc + 1) * QC)
            nkc = 4 * (qc + 1)
            sum_t = srpool.tile([NH, QC], F32, tag="sum")
            rcp_t = srpool.tile([NH, QC], F32R, tag="rcp")
            for j in range(NPAIR):
                # interleave the previous chunk's normalization + output
                # projection into this chunk's exp-bound attention.  All
                # four pairs must be normalized before any projection group
                # (each group contracts over every pair).
                if pending is not None:
                    pqc, prcp = pending
                    # norms start at pair 1 so pair 0's scores can fill the
                    # reciprocal wait at the chunk boundary
                    if j == 1:
                        for pj in range(NPAIR):
                            emit_norm_oproj(pqc, prcp, pj)
                    if j >= 1:
                        for no in range(D // QC):
                            emit_oproj_group(pqc, j - 1, no)
                ot0 = otps.tile([DK + 1, QC], F32, tag="ot0")
                ot1 = otps.tile([DK + 1, QC], F32, tag="ot1")
                for kc in range(nkc):
                    diag = kc >= 4 * qc
                    qlo = (kc - 4 * qc) * P if diag else 0
                    qs = slice(qc * QC + qlo, (qc + 1) * QC)
                    # both heads' scores side by side in one 2-bank psum tile
                    sp = spps.tile([P, 2 * QC], F32, tag="sp")
                    nc.tensor.matmul(
                        sp[:, qlo:QC], kt_sb[0:DK, j, kc * P:(kc + 1) * P],
                        qt_sb[0:DK, j, qs], start=True, stop=True)
                    nc.tensor.matmul(
                        sp[:, QC + qlo:], kt_sb[DK:P, j, kc * P:(kc + 1) * P],
                        qt_sb[DK:P, j, qs], start=True, stop=True)
                    pt = ptpool.tile([P, 2 * QC], F32R, tag="pt")
                    # one exp over both heads' (possibly narrowed) ranges
                    pt2 = pt.rearrange("p (a b) -> p a b", a=2)
                    sp2 = sp.rearrange("p (a b) -> p a b", a=2)
                    nc.scalar.activation(pt2[:, :, qlo:], sp2[:, :, qlo:], EXP)
                    if diag:
                        nc.vector.tensor_mul(
                            pt[:, qlo:qlo + P], pt[:, qlo:qlo + P], tri_sb[:])
                        nc.vector.tensor_mul(
                            pt[:, QC + qlo:QC + qlo + P],
                            pt[:, QC + qlo:QC + qlo + P], tri_sb[:])
                    nc.tensor.matmul(
                        ot0[:, qlo:], v_sb[:, kc, 2 * j, :], pt[:, qlo:QC],
                        start=(kc == 0), stop=(kc == nkc - 1),
                        skip_group_check=True)
                    nc.tensor.matmul(
                        ot1[:, qlo:], v_sb[:, kc, 2 * j + 1, :], pt[:, QC + qlo:],
                        start=(kc == 0), stop=(kc == nkc - 1),
                        skip_group_check=True)
                # drain: even head's context straight to its rows; the odd
                # head (and both denominator rows) bounce through SBUF and
                # DMA to their partition-shifted slots.
                bn0 = bnpool.tile([DK + 1, QC], F32R, tag="bn0")
                bn1 = bnpool.tile([DK + 1, QC], F32R, tag="bn1")
                tail = qc == NQC - 1 and j == NPAIR - 1
                if tail:
                    # final pair: denominator rows first (they gate the
                    # closing reciprocal -> normalize -> project chain)
                    nc.vector.tensor_copy(bn0[DK:DK + 1, :],
                                          ot0[DK:DK + 1, :])
                    nc.vector.tensor_copy(bn1[:], ot1[:])
                    nc.vector.tensor_copy(ctx_sb[0:DK, j, qc, :], ot0[0:DK, :])
                else:
                    nc.vector.tensor_copy(ctx_sb[0:DK, j, qc, :], ot0[0:DK, :])
                    nc.vector.tensor_copy(bn0[DK:DK + 1, :],
                                          ot0[DK:DK + 1, :])
                    nc.vector.tensor_copy(bn1[:], ot1[:])
                nc.gpsimd.dma_start(ctx_sb[DK:P, j, qc, :], bn1[0:DK, :])
                # final pair: denominators on idle queues, in parallel
                e0 = nc.scalar if tail else nc.gpsimd
                e1 = nc.sync if tail else nc.gpsimd
                e0.dma_start(sum_t[2 * j:2 * j + 1, :],
                             bn0[DK:DK + 1, :].bitcast(F32))
                e1.dma_start(sum_t[2 * j + 1:2 * j + 2, :],
                             bn1[DK:DK + 1, :].bitcast(F32))

            # the held-back projection block lands here, filling this
            # chunk's own drain waits
            if pending is not None:
                for no in range(D // QC):
                    emit_oproj_group(pending[0], NPAIR - 1, no)
            # lazy V projection for the NEXT q chunk's new k-window;
            # streams x back in from DRAM (x_sb was released after phase 1)
            if qc < NQC - 1:
                for sc in range(4 * (qc + 1), 4 * (qc + 2)):
                    xv = xvpool.tile([P, KO, P], F32R, tag="xv")
                    nc.sync.dma_start(xv[:], xt_r[:, :, sc * P:(sc + 1) * P])
                    vps = mixps.tile([P, HD], F32, tag="mix")
                    for ko in range(KO):
                        nc.tensor.matmul(
                            vps[:],
                            xv[:, ko, :],
                            wv2_sb[:, ko, :],
                            start=(ko == 0), stop=(ko == KO - 1),
                        )
                    nc.vector.tensor_copy(
                        v_sb[:, sc, :, 0:DK],
                        vps.rearrange("p (h e) -> p h e", h=NH))

            # reciprocal of this chunk's denominators; norm + O-proj are
            # deferred into the next chunk's pair loop (PE filler there)
            with nc.allow_low_precision(reason="f32r rounding of 1/denom"):
                nc.vector.reciprocal(rcp_t[:], sum_t[:])
            if qc < NQC - 1:
                pending = (qc, rcp_t)
            else:
                for j in range(NPAIR):
                    emit_norm_oproj(qc, rcp_t, j)
                for t in range(QC // P):
                    for no in range(D // QC):
                        emit_oproj_group(qc, t, no)


def build_nc():
    nc = bacc.Bacc("TRN2", target_bir_lowering=False, debug=False)
    xt = nc.dram_tensor("xt", [D, S], F32R, kind="ExternalInput").ap()
    wq = nc.dram_tensor("wq", [D, HD], F32R, kind="ExternalInput").ap()
    wk = nc.dram_tensor("wk", [D, HD], F32R, kind="ExternalInput").ap()
    wv = nc.dram_tensor("wv", [D, HD], F32R, kind="ExternalInput").ap()
    wo = nc.dram_tensor("wo", [HD, D], F32R, kind="ExternalInput").ap()
    tri = nc.dram_tensor("tri", [P, P], F32R, kind="ExternalInput").ap()
    m2 = nc.dram_tensor("m2", [NH, NPAIR * P], F32R, kind="ExternalInput").ap()
    ones = nc.dram_tensor("ones", [P, NKC * NH], F32R, kind="ExternalInput").ap()
    out = nc.dram_tensor("out", [S, D], F32, kind="ExternalOutput").ap()
    with tile.TileContext(nc) as tc:
        with ExitStack() as ctx:
            _emit(ctx, tc, xt, wq, wk, wv, wo, tri, m2, ones, out)
    nc.compile()
    return nc


def make_in_maps(x, W_q, W_k, W_v, W_o):
    x = np.asarray(x, dtype=np.float32)
    WqT = np.ascontiguousarray(np.asarray(W_q, np.float32).T)
    WkT = np.ascontiguousarray(np.asarray(W_k, np.float32).T)
    WvT = np.ascontiguousarray(np.asarray(W_v, np.float32).T)
    WoT = np.ascontiguousarray(np.asarray(W_o, np.float32).T)
    tri = np.triu(np.ones((P, P), np.float32))  # tri[k,q] = 1 where q >= k
    m2 = np.zeros((NH, NPAIR * P), np.float32)
    for j in range(NPAIR):
        m2[2 * j, j * P:j * P + DK] = 1.0
        m2[2 * j + 1, j * P + DK:(j + 1) * P] = 1.0
    in_maps = []
    for c in range(2 * B):
        b, g = c // 2, c % 2
        in_maps.append({
            "xt": np.ascontiguousarray(x[b].T),
            "wq": np.ascontiguousarray(WqT[:, g * HD:(g + 1) * HD]),
            "wk": np.ascontiguousarray(WkT[:, g * HD:(g + 1) * HD]),
            "wv": np.ascontiguousarray(WvT[:, g * HD:(g + 1) * HD]),
            "wo": np.ascontiguousarray(WoT[g * HD:(g + 1) * HD, :]),
            "tri": tri,
            "m2": m2,
            "ones": np.ones((P, NKC * NH), np.float32),
        })
    return in_maps


def get_runner():
    """Build (once) and cache a jitted 8-core executor for the bass program.

    Returns run(in_maps) -> list of per-core {name: np.ndarray} outputs.
    Mirrors concourse.bass2jax.run_bass_via_pjrt but caches the jitted
    callable so repeat kernel() calls skip re-lowering/compiling.
    """
    if "runner" in _CACHE:
        return _CACHE["runner"]
    import jax
    from jax.experimental.shard_map import shard_map
    from jax.sharding import Mesh, PartitionSpec
    from concourse import mybir as _mb
    from concourse.bass2jax import (
        _bass_exec_p, install_neuronx_cc_hook, partition_id_tensor)

    install_neuronx_cc_hook()
    nc = build_nc()
    n_cores = 2 * B

    partition_name = (nc.partition_id_tensor.name
                      if nc.partition_id_tensor else None)
    in_names, out_names, out_avals = [], [], []
    for alloc in nc.m.functions[0].allocations:
        if not isinstance(alloc, _mb.MemoryLocationSet):
            continue
        name = alloc.memorylocations[0].name
        if alloc.kind == "ExternalInput":
            if name != partition_name:
                in_names.append(name)
        elif alloc.kind == "ExternalOutput":
            out_names.append(name)
            out_avals.append(jax.core.ShapedArray(
                tuple(alloc.tensor_shape), _mb.dt.np(alloc.dtype)))
    n_params = len(in_names)
    all_names = in_names + out_names
    if partition_name is not None:
        all_names = all_names + [partition_name]

    def _body(*args):
        operands = list(args)
        if partition_name is not None:
            operands.append(partition_id_tensor())
        outs = _bass_exec_p.bind(
            *operands,
            out_avals=tuple(out_avals),
            in_names=tuple(all_names),
            out_names=tuple(out_names),
            lowering_input_output_aliases=(),
            sim_require_finite=True,
            sim_require_nnan=True,
            nc=nc,
        )
        return tuple(outs)

    devices = jax.devices()[:n_cores]
    mesh = Mesh(np.asarray(devices), ("core",))
    n_outs = len(out_names)
    sharded = jax.jit(
        shard_map(
            _body, mesh=mesh,
            in_specs=(PartitionSpec("core"),) * (n_params + n_outs),
            out_specs=(PartitionSpec("core"),) * n_outs,
            check_rep=False,
        ),
        donate_argnums=tuple(range(n_params, n_params + n_outs)),
        keep_unused=True,
    )

    def run(in_maps, device_arrays=None):
        concat_in = device_arrays if device_arrays is not None else [
            np.concatenate([np.asarray(in_maps[c][i_name])
                            for c in range(n_cores)], axis=0)
            for i_name in in_names
        ]
        concat_zeros = [
            np.zeros((n_cores * av.shape[0], *av.shape[1:]), av.dtype)
            for av in out_avals
        ]
        out_arrs = sharded(*concat_in, *concat_zeros)
        return [
            {name: np.asarray(out_arrs[i]).reshape(
                n_cores, *out_avals[i].shape)[c]
             for i, name in enumerate(out_names)}
            for c in range(n_cores)
        ]

    _CACHE["runner"] = (run, in_names, out_avals)
    return _CACHE["runner"]


def _run_cores(in_maps):
    """Execute the 8-core program; returns per-core {name: np.ndarray}."""
    from concourse._compat import axon_active
    if axon_active():
        # remote-accelerator proxy: use the cached jitted PJRT executor so
        # repeat calls skip re-lowering/compiling
        run, _, _ = get_runner()
        return run(in_maps)
    # native path (local /dev/neuron*): run_bass_kernel_spmd handles NEFF
    # compile caching + device execution directly
    if "nc" not in _CACHE:
        _CACHE["nc"] = build_nc()
    res = run_bass_kernel_spmd(_CACHE["nc"], in_maps, core_ids=list(range(2 * B)))
    _CACHE["last_exec_time_ns"] = res.exec_time_ns
    return res.results


def kernel(x, W_q, W_k, W_v, W_o):
    in_maps = make_in_maps(x, W_q, W_k, W_v, W_o)
    results = _run_cores(in_maps)
    out = np.empty((B, S, D), np.float32)
    for b in range(B):
        out[b] = results[2 * b]["out"] + results[2 * b + 1]["out"]
    return out



# revision 24
# speedup vs baseline: 1.1999x; 1.1999x over previous
"""Multi-head causal self-attention on 8 Trainium2 NeuronCores.

Problem: B=4, S=2048, D=1024, H=16 heads (dk=64), fp32 in/out, causal softmax.

Sharding: hybrid batch x head-group. Core c handles batch b = c//2 and head
group g = c%2 (8 heads = 512 dims). Each core computes QKV projections for
its head group, causal attention, and a partial output projection over its
512 context dims. The host sums the two bf16 partials per batch in fp32.

Device-side design (per core), all matmul operands bf16 (PSUM fp32):
  - Cost model charges matmuls by output free-size only, so every matmul is
    arranged to produce 128 output partitions per moving row where possible.
  - Q^T/K^T [128, pair, S]: partition block = head pair (64 rows each head).
  - Scores S^T[k, q] per (q-tile 128, k-chunk 128): 8 matmuls (contraction
    dk=64) into one 2-bank PSUM tile [128, 8head, 128q]; one exp (ACT) over
    all 8 heads; triangular mask multiply on the diagonal chunk only.
  - P@V runs TRANSPOSED: out ctx[q 128, 8, dv 64+1] with lhsT = exp(S^T)
    [k,q] and rhs = V_aug [k, 65] (ones column -> softmax denominator), so
    each PV matmul moves only 65 rows instead of 512.  The ctx accumulator
    PSUM tile is memset-zeroed once per q-tile and all PV matmuls use
    start=False (multiple interleaved accumulation groups share banks; the
    2KB zeroing granularity of start=True would clobber neighbours).
  - Normalization is a per-partition broadcast multiply on DVE (q is the
    partition dim), then one PE transpose per head PAIR ([q,2*64] -> pair
    layout [128, q]) readies ctx^T as the lhsT of the output projection.
  - Output projection accumulates 4 pairs x 512 cols in PSUM; bf16 partial
    written to DRAM; host adds the two head-group partials per batch.
  - Schedule: forward q-tile sweep with K/Q/V projection groups and
    deferred output projections interleaved into the attention loops as PE
    filler under the exp-bound (ACT) stretches.
"""

import numpy as np
from contextlib import ExitStack

import concourse.bass as bass
import concourse.tile as tile
from concourse import bacc, mybir
from concourse.bass_utils import run_bass_kernel_spmd

B, S, D = 4, 2048, 1024
H16 = 16
DK = 64
G = 2               # head groups (cores per batch)
HD = D // G         # per-core head dims = 512 (8 heads)
NH = HD // DK       # heads per core = 8
NJ = NH // 2        # head pairs per core = 4
P = 128
NQS = S // P        # 16 q subtiles
NKC = S // P        # 16 k chunks
KO = D // P         # 8 contraction chunks for projections
QC = 512            # projection s-chunk

F32 = mybir.dt.float32
BF16 = mybir.dt.bfloat16
EXP = mybir.ActivationFunctionType.Exp

_CACHE: dict = {}


def _emit(ctx: ExitStack, tc, xt, wq, wk, wv, wo, tri, ident, out):
    nc = tc.nc

    persist = ctx.enter_context(tc.tile_pool(name="persist", bufs=1))
    xt_sb = persist.tile([P, KO, S], BF16)
    wq_sb = persist.tile([P, KO, HD], BF16)
    wk_sb = persist.tile([P, KO, HD], BF16)
    wv_sb = persist.tile([P, KO, HD], BF16)
    wo_sb = persist.tile([P, NJ, D], BF16)
    qt_ev = persist.tile([P, NJ, S], BF16)
    qt_od = persist.tile([P, NJ, S], BF16)
    kt_sb = persist.tile([P, NJ, S], BF16)
    v_sb = persist.tile([P, NKC, NH, DK + 1], BF16)
    ctxT_sb = persist.tile([P, NJ, NQS, P], BF16)
    tri_sb = persist.tile([P, 1, P], BF16)
    id_sb = persist.tile([P, P], BF16)
    warm_sb = persist.tile([P, 2], BF16)

    xt_r = xt.rearrange("(o p) s -> p o s", p=P)
    wq_r = wq.rearrange("(o p) m -> p o m", p=P)
    wk_r = wk.rearrange("(o p) m -> p o m", p=P)
    wv_r = wv.rearrange("(o p) m -> p o m", p=P)
    wo_r = wo.rearrange("(j p) o -> p j o", p=P)

    with (
        tc.tile_pool(name="spps", bufs=2, space="PSUM") as spps,
        tc.tile_pool(name="cxps", bufs=1, space="PSUM") as cxps,
        tc.tile_pool(name="mixps", bufs=2, space="PSUM") as mixps,
        tc.tile_pool(name="ptp", bufs=3) as ptp,
        tc.tile_pool(name="cnp", bufs=2) as cnp,
        tc.tile_pool(name="rrp", bufs=2) as rrp,
        tc.tile_pool(name="osb", bufs=3) as osb,
    ):
        # ---- input DMAs, interleaved so the first K-proj group is fully fed
        # by ~2.5us.  The prologue needs only wk + the first s-chunk of x, so
        # x's sc0 columns load first, the remainder streams in behind.
        # queues: SP wk/consts/wq/wo; Pool + ACT split x chunks; Pool wv.
        for ko in range(4):
            nc.sync.dma_start(wk_sb[:, ko, :], wk_r[:, ko, :])
        for ko in range(0, KO, 2):
            nc.gpsimd.dma_start(xt_sb[:, ko, 0:QC], xt_r[:, ko, 0:QC])
            nc.scalar.dma_start(xt_sb[:, ko + 1, 0:QC], xt_r[:, ko + 1, 0:QC])
        for ko in range(4, KO):
            nc.gpsimd.dma_start(wk_sb[:, ko, :], wk_r[:, ko, :])
        nc.sync.dma_start(id_sb[:], ident)
        nc.sync.dma_start(tri_sb[:, 0, :], tri)
        # exp activation-table warm-up: off the critical path
        nc.scalar.activation(warm_sb[:], id_sb[:, 0:2], EXP)
        for ko in range(KO):
            nc.sync.dma_start(wq_sb[:, ko, :], wq_r[:, ko, :])
            eng = nc.gpsimd if ko % 2 == 0 else nc.scalar
            eng.dma_start(xt_sb[:, ko, QC:S], xt_r[:, ko, QC:S])
        for ko in range(KO):
            nc.gpsimd.dma_start(wv_sb[:, ko, :], wv_r[:, ko, :])
        nc.sync.dma_start(wo_sb[:], wo_r)
        # ones column of V_aug (softmax denominator accumulator)
        nc.gpsimd.memset(v_sb[:, :, :, DK : DK + 1], 1.0)
        # Q is stored twice with the other head's rows zeroed so score
        # matmuls contract the full 128 partitions from base partition 0
        # (operands at base partition 64 fail on hardware)
        nc.gpsimd.memset(qt_ev[DK:P, :, :], 0.0)
        nc.gpsimd.memset(qt_od[0:DK, :, :], 0.0)

        # ---- building blocks ------------------------------------------
        def drain_k(pj, j, sc):
            nc.vector.tensor_copy(kt_sb[:, j, sc * QC : (sc + 1) * QC], pj[:])

        def drain_q(pj, j, sc):
            nc.vector.tensor_copy(
                qt_ev[0:DK, j, sc * QC : (sc + 1) * QC], pj[0:DK, :])
            nc.vector.tensor_copy(
                qt_od[DK:P, j, sc * QC : (sc + 1) * QC], pj[DK:P, :])

        def emit_kq_group(drain, w_sb, j, sc):
            pj = mixps.tile([P, QC], F32, tag="mix", name="pj")
            for ko in range(KO):
                nc.tensor.matmul(
                    pj[:],
                    w_sb[:, ko, j * P : (j + 1) * P],
                    xt_sb[:, ko, sc * QC : (sc + 1) * QC],
                    start=(ko == 0),
                    stop=(ko == KO - 1),
                )
            drain(pj, j, sc)

        def emit_v_group(kc):
            vp = mixps.tile([P, HD], F32, tag="mix", name="vp")
            for ko in range(KO):
                nc.tensor.matmul(
                    vp[:],
                    xt_sb[:, ko, kc * P : (kc + 1) * P],
                    wv_sb[:, ko, :],
                    start=(ko == 0),
                    stop=(ko == KO - 1),
                )
            nc.vector.tensor_copy(
                v_sb[:, kc, :, 0:DK], vp.rearrange("p (h e) -> p h e", h=NH)
            )

        def emit_oproj_half(qs, no):
            op = mixps.tile([P, QC], F32, tag="mix", name="op")
            for j in range(NJ):
                nc.tensor.matmul(
                    op[:],
                    ctxT_sb[:, j, qs, :],
                    wo_sb[:, j, no * QC : (no + 1) * QC],
                    start=(j == 0),
                    stop=(j == NJ - 1),
                )
            o2 = osb.tile([P, QC], BF16, tag="o")
            nc.vector.tensor_copy(o2[:], op[:])
            nc.sync.dma_start(
                out[qs * P : (qs + 1) * P, no * QC : (no + 1) * QC], o2[:]
            )

        def emit_oproj(qs):
            for no in range(2):
                emit_oproj_half(qs, no)

        # ---- prologue: first-chunk projections -------------------------
        for j in range(NJ):
            emit_kq_group(drain_k, wk_sb, j, 0)
        for j in range(NJ):
            emit_kq_group(drain_q, wq_sb, j, 0)
        emit_v_group(0)

        # PE filler emitted inside the attention loops: later chunks'
        # projections plus deferred early output projections, broken into
        # ~0.4us micro-steps (2 matmuls each) so one step per kc exactly
        # covers the PE deficit under the exp-bound (ACT 1.04us/kc) stretches.
        # Each (deadline, thunk) step must run before attn(deadline) starts.
        from collections import deque

        def kq_steps(drain_fn, w_sb, j, sc, dl):
            st = {}

            def mk(k0):
                def f():
                    if k0 == 0:
                        st["pj"] = mixps.tile([P, QC], F32, tag="mix", name="pj")
                    for ko in (k0, k0 + 1):
                        nc.tensor.matmul(
                            st["pj"][:],
                            w_sb[:, ko, j * P : (j + 1) * P],
                            xt_sb[:, ko, sc * QC : (sc + 1) * QC],
                            start=(ko == 0),
                            stop=(ko == KO - 1),
                        )
                return f

            def drain():
                drain_fn(st["pj"], j, sc)

            return [(dl, mk(k)) for k in (0, 2, 4, 6)] + [(dl, drain)]

        def v_steps(kc, dl):
            st = {}

            def mk(k0):
                def f():
                    if k0 == 0:
                        st["vp"] = mixps.tile([P, HD], F32, tag="mix", name="vp")
                    for ko in (k0, k0 + 1):
                        nc.tensor.matmul(
                            st["vp"][:],
                            xt_sb[:, ko, kc * P : (kc + 1) * P],
                            wv_sb[:, ko, :],
                            start=(ko == 0),
                            stop=(ko == KO - 1),
                        )
                return f

            def drain():
                nc.vector.tensor_copy(
                    v_sb[:, kc, :, 0:DK],
                    st["vp"].rearrange("p (h e) -> p h e", h=NH))

            return [(dl, mk(k)) for k in (0, 2, 4, 6)] + [(dl, drain)]

        def op_steps(oqs, no, dl):
            st = {}

            def mk(j0):
                def f():
                    if j0 == 0:
                        st["op"] = mixps.tile([P, QC], F32, tag="mix", name="op")
                    for j in (j0, j0 + 1):
                        nc.tensor.matmul(
                            st["op"][:],
                            ctxT_sb[:, j, oqs, :],
                            wo_sb[:, j, no * QC : (no + 1) * QC],
                            start=(j == 0),
                            stop=(j == NJ - 1),
                        )
                return f

            def drain():
                o2 = osb.tile([P, QC], BF16, tag="o", name="o2")
                nc.vector.tensor_copy(o2[:], st["op"][:])
                nc.sync.dma_start(
                    out[oqs * P : (oqs + 1) * P, no * QC : (no + 1) * QC],
                    o2[:])

            return [(dl, mk(0)), (dl, mk(2)), (dl, drain)]

        steps = deque()
        for sc in (1, 2, 3):
            lo = 4 * sc
            for kc in range(lo - 3, lo):
                steps.extend(v_steps(kc, kc))
            for j in range(NJ):
                steps.extend(kq_steps(drain_k, wk_sb, j, sc, lo))
            for j in range(NJ):
                steps.extend(kq_steps(drain_q, wq_sb, j, sc, lo))
            steps.extend(v_steps(lo, lo))
        for kc in (13, 14, 15):
            steps.extend(v_steps(kc, kc))
        for oqs in range(5):  # OP(5) held back as tail filler
            steps.extend(op_steps(oqs, 0, 99))
            steps.extend(op_steps(oqs, 1, 99))

        tri_b = tri_sb.broadcast_to([P, NH, P])

        def emit_norm(qs, cx4):
            # normalize (q is the partition dim -> broadcast along free)
            rr = rrp.tile([P, 2, 4, 1], F32, tag="rr", name="rr")
            nc.vector.reciprocal(rr[:], cx4[:, :, :, DK : DK + 1])
            cn = cnp.tile([P, 2, 4, DK], BF16, tag="cn", name="cn")
            nc.vector.tensor_mul(
                cn[:], cx4[:, :, :, 0:DK], rr.broadcast_to([P, 2, 4, DK]))
            return cn

        def emit_transp(qs, cn):
            # transpose to pair layout: one PE transpose per head pair
            tp = mixps.tile([P, NJ, P], BF16, tag="mix", name="tp")
            for j in range(NJ):
                nc.tensor.matmul(
                    tp[:, j, :],
                    cn[:, (2 * j) // 4, (2 * j) % 4 : (2 * j) % 4 + 2, :],
                    id_sb[:],
                    is_transpose=True,
                    start=True,
                    stop=True,
                    skip_group_check=True,
                )
            nc.vector.tensor_copy(ctxT_sb[:, :, qs, :], tp[:])

        # ---- main q-tile sweep ------------------------------------------
        # The norm/transpose/oproj of q-tile qs is deferred into the first kc
        # steps of qs+1, so PE never waits on the DVE norm chain at a q-tile
        # boundary: the next tile's scores run under it.
        import os
        nqs_lim = int(os.environ.get("NQS_LIM", NQS))
        pending = None  # (qs, cx4) awaiting norm/transpose/output-projection
        for qs in range(nqs_lim):
            nkc = qs + 1
            pcn = None
            if pending is not None:
                pcn = emit_norm(pending[0], pending[1])  # DVE only
            # 4 heads per 512-f32 PSUM bank (65-wide groups must not cross a
            # bank boundary): head h lives at cx4[:, h//4, h%4, :]
            cx = cxps.tile([P, 2, 512], F32, tag="cx", name="cx")
            cx4 = cx[:, :, 0 : 4 * (DK + 1)].rearrange(
                "p b (h e) -> p b h e", h=4)
            nc.vector.memset(cx4[:], 0.0)
            # force any filler whose deadline has arrived
            while steps and steps[0][0] <= qs:
                steps.popleft()[1]()
            for kc in range(nkc):
                sp = spps.tile([P, NH, P], F32, tag="sp", name="sp")
                for h in range(NH):
                    j = h // 2
                    qsrc = qt_ev if h % 2 == 0 else qt_od
                    nc.tensor.matmul(
                        sp[:, h, :],
                        kt_sb[:, j, kc * P : (kc + 1) * P],
                        qsrc[:, j, qs * P : (qs + 1) * P],
                        start=True,
                        stop=True,
                        skip_group_check=True,
                    )
                pt = ptp.tile([P, NH, P], BF16, tag="pt", name="pt")
                nc.scalar.activation(pt[:], sp[:], EXP)
                if kc == qs:
                    nc.vector.tensor_mul(pt[:], pt[:], tri_b)
                if pending is not None:
                    # place the deferred transpose/oproj deep enough into this
                    # tile's kc steps that the DVE norm chain and the ctxT
                    # copy complete under preceding PE work
                    if kc == min(1, nkc - 1):
                        emit_transp(pending[0], pcn)
                    if pending[0] >= 6:
                        if kc == 2:
                            emit_oproj_half(pending[0], 0)
                        if kc == 3:
                            emit_oproj_half(pending[0], 1)
                # filler micro-steps sized to this kc step's PE deficit under
                # the exp rate: extra at the tile boundary (kc 0), none where
                # the inline oproj halves already fill (kc 2-3)
                if kc == 0:
                    want = 2
                elif kc in (2, 3) and pending is not None and pending[0] >= 6:
                    want = 0
                else:
                    want = 1
                for _ in range(want):
                    if steps:
                        steps.popleft()[1]()
                for h in range(NH):
                    nc.tensor.matmul(
                        cx4[:, h // 4, h % 4, :],
                        pt[:, h, :],
                        v_sb[:, kc, h, :],
                        start=False,
                        stop=(kc == nkc - 1),
                        skip_group_check=True,
                    )
            pending = (qs, cx4)
        # tail: last q-tile's norm/transpose/projection, with the held-back
        # OP(5) (plus any queue remainder) giving PE work while the DVE norm
        # chain and ctxT copy land
        if pending is not None and nqs_lim == NQS:
            cn15 = emit_norm(pending[0], pending[1])
            while steps:
                steps.popleft()[1]()
            emit_oproj_half(5, 0)
            emit_transp(pending[0], cn15)
            emit_oproj_half(5, 1)
            emit_oproj(pending[0])


def build_nc():
    nc = bacc.Bacc("TRN2", target_bir_lowering=False, debug=False)
    xt = nc.dram_tensor("xt", [D, S], BF16, kind="ExternalInput").ap()
    wq = nc.dram_tensor("wq", [D, HD], BF16, kind="ExternalInput").ap()
    wk = nc.dram_tensor("wk", [D, HD], BF16, kind="ExternalInput").ap()
    wv = nc.dram_tensor("wv", [D, HD], BF16, kind="ExternalInput").ap()
    wo = nc.dram_tensor("wo", [HD, D], BF16, kind="ExternalInput").ap()
    tri = nc.dram_tensor("tri", [P, P], BF16, kind="ExternalInput").ap()
    ident = nc.dram_tensor("ident", [P, P], BF16, kind="ExternalInput").ap()
    out = nc.dram_tensor("out", [S, D], BF16, kind="ExternalOutput").ap()
    with tile.TileContext(nc) as tc:
        with ExitStack() as ctx:
            with nc.allow_low_precision(reason="bf16 kernel by design"):
                _emit(ctx, tc, xt, wq, wk, wv, wo, tri, ident, out)
    nc.compile()
    return nc


def make_in_maps(x, W_q, W_k, W_v, W_o):
    import ml_dtypes

    BF = ml_dtypes.bfloat16
    x = np.asarray(x, dtype=np.float32)
    # fold the 1/sqrt(dk)=1/8 softmax scale into W_q (exact power of two)
    WqT = np.ascontiguousarray(np.asarray(W_q, np.float32).T * 0.125).astype(BF)
    WkT = np.ascontiguousarray(np.asarray(W_k, np.float32).T).astype(BF)
    WvT = np.ascontiguousarray(np.asarray(W_v, np.float32).T).astype(BF)
    WoT = np.ascontiguousarray(np.asarray(W_o, np.float32).T).astype(BF)
    # tri[k, q] = 1 where q >= k (within a diagonal 128x128 block)
    tri = np.triu(np.ones((P, P), np.float32)).astype(BF)
    ident = np.eye(P, dtype=np.float32).astype(BF)
    in_maps = []
    for c in range(2 * B):
        b, g = c // 2, c % 2
        in_maps.append({
            "xt": np.ascontiguousarray(x[b].T).astype(BF),
            "wq": np.ascontiguousarray(WqT[:, g * HD : (g + 1) * HD]),
            "wk": np.ascontiguousarray(WkT[:, g * HD : (g + 1) * HD]),
            "wv": np.ascontiguousarray(WvT[:, g * HD : (g + 1) * HD]),
            "wo": np.ascontiguousarray(WoT[g * HD : (g + 1) * HD, :]),
            "tri": tri,
            "ident": ident,
        })
    return in_maps


def get_runner():
    """Build (once) and cache a jitted 8-core executor for the bass program.

    Returns run(in_maps) -> list of per-core {name: np.ndarray} outputs.
    Mirrors concourse.bass2jax.run_bass_via_pjrt but caches the jitted
    callable so repeat kernel() calls skip re-lowering/compiling.
    """
    if "runner" in _CACHE:
        return _CACHE["runner"]
    import jax
    from jax.experimental.shard_map import shard_map
    from jax.sharding import Mesh, PartitionSpec
    from concourse import mybir as _mb
    from concourse.bass2jax import (
        _bass_exec_p, install_neuronx_cc_hook, partition_id_tensor)

    install_neuronx_cc_hook()
    nc = build_nc()
    n_cores = 2 * B

    partition_name = (nc.partition_id_tensor.name
                      if nc.partition_id_tensor else None)
    in_names, out_names, out_avals = [], [], []
    for alloc in nc.m.functions[0].allocations:
        if not isinstance(alloc, _mb.MemoryLocationSet):
            continue
        name = alloc.memorylocations[0].name
        if alloc.kind == "ExternalInput":
            if name != partition_name:
                in_names.append(name)
        elif alloc.kind == "ExternalOutput":
            out_names.append(name)
            out_avals.append(jax.core.ShapedArray(
                tuple(alloc.tensor_shape), _mb.dt.np(alloc.dtype)))
    n_params = len(in_names)
    all_names = in_names + out_names
    if partition_name is not None:
        all_names = all_names + [partition_name]

    def _body(*args):
        operands = list(args)
        if partition_name is not None:
            operands.append(partition_id_tensor())
        outs = _bass_exec_p.bind(
            *operands,
            out_avals=tuple(out_avals),
            in_names=tuple(all_names),
            out_names=tuple(out_names),
            lowering_input_output_aliases=(),
            sim_require_finite=True,
            sim_require_nnan=True,
            nc=nc,
        )
        return tuple(outs)

    devices = jax.devices()[:n_cores]
    mesh = Mesh(np.asarray(devices), ("core",))
    n_outs = len(out_names)
    sharded = jax.jit(
        shard_map(
            _body, mesh=mesh,
            in_specs=(PartitionSpec("core"),) * (n_params + n_outs),
            out_specs=(PartitionSpec("core"),) * n_outs,
            check_rep=False,
        ),
        donate_argnums=tuple(range(n_params, n_params + n_outs)),
        keep_unused=True,
    )

    def run(in_maps, device_arrays=None):
        concat_in = device_arrays if device_arrays is not None else [
            np.concatenate([np.asarray(in_maps[c][i_name])
                            for c in range(n_cores)], axis=0)
            for i_name in in_names
        ]
        concat_zeros = [
            np.zeros((n_cores * av.shape[0], *av.shape[1:]), av.dtype)
            for av in out_avals
        ]
        out_arrs = sharded(*concat_in, *concat_zeros)
        return [
            {name: np.asarray(out_arrs[i]).reshape(
                n_cores, *out_avals[i].shape)[c]
             for i, name in enumerate(out_names)}
            for c in range(n_cores)
        ]

    _CACHE["runner"] = (run, in_names, out_avals)
    return _CACHE["runner"]


def _run_cores(in_maps):
    """Execute the 8-core program; returns per-core {name: np.ndarray}."""
    from concourse._compat import axon_active
    if axon_active():
        # remote-accelerator proxy: use the cached jitted PJRT executor so
        # repeat calls skip re-lowering/compiling
        run, _, _ = get_runner()
        return run(in_maps)
    # native path (local /dev/neuron*): run_bass_kernel_spmd handles NEFF
    # compile caching + device execution directly
    if "nc" not in _CACHE:
        _CACHE["nc"] = build_nc()
    res = run_bass_kernel_spmd(_CACHE["nc"], in_maps, core_ids=list(range(2 * B)))
    _CACHE["last_exec_time_ns"] = res.exec_time_ns
    return res.results


def kernel(x, W_q, W_k, W_v, W_o):
    in_maps = make_in_maps(x, W_q, W_k, W_v, W_o)
    results = _run_cores(in_maps)
    out = np.empty((B, S, D), np.float32)
    for b in range(B):
        out[b] = (results[2 * b]["out"].astype(np.float32)
                  + results[2 * b + 1]["out"].astype(np.float32))
    return out


# revision 25
# speedup vs baseline: 1.2326x; 1.0272x over previous
"""Multi-head causal self-attention on 8 Trainium2 NeuronCores.

Problem: B=4, S=2048, D=1024, H=16 heads (dk=64), fp32 in/out, causal softmax.

Sharding: hybrid batch x head-group. Core c handles batch b = c//2 and head
group g = c%2 (8 heads = 512 dims). Each core computes QKV projections for
its head group, causal attention, and a partial output projection over its
512 context dims. The host sums the two bf16 partials per batch in fp32.

Device-side design (per core), all matmul operands bf16 (PSUM fp32):
  - Cost model charges matmuls by output free-size only, so every matmul is
    arranged to produce 128 output partitions per moving row where possible.
  - Q^T/K^T [128, pair, S]: partition block = head pair (64 rows each head).
  - Scores S^T[k, q] per (q-tile 128, k-chunk 128): 8 matmuls (contraction
    dk=64) into one 2-bank PSUM tile [128, 8head, 128q]; one exp (ACT) over
    all 8 heads; triangular mask multiply on the diagonal chunk only.
  - P@V runs TRANSPOSED: out ctx[q 128, 8, dv 64+1] with lhsT = exp(S^T)
    [k,q] and rhs = V_aug [k, 65] (ones column -> softmax denominator), so
    each PV matmul moves only 65 rows instead of 512.  The ctx accumulator
    PSUM tile is memset-zeroed once per q-tile and all PV matmuls use
    start=False (multiple interleaved accumulation groups share banks; the
    2KB zeroing granularity of start=True would clobber neighbours).
  - Normalization is a per-partition broadcast multiply on DVE (q is the
    partition dim), then one PE transpose per head PAIR ([q,2*64] -> pair
    layout [128, q]) readies ctx^T as the lhsT of the output projection.
  - Output projection accumulates 4 pairs x 512 cols in PSUM; bf16 partial
    written to DRAM; host adds the two head-group partials per batch.
  - Schedule: forward q-tile sweep with K/Q/V projection groups and
    deferred output projections interleaved into the attention loops as PE
    filler under the exp-bound (ACT) stretches.
"""

import numpy as np
from contextlib import ExitStack

import concourse.bass as bass
import concourse.tile as tile
from concourse import bacc, mybir
from concourse.bass_utils import run_bass_kernel_spmd

B, S, D = 4, 2048, 1024
H16 = 16
DK = 64
G = 2               # head groups (cores per batch)
HD = D // G         # per-core head dims = 512 (8 heads)
NH = HD // DK       # heads per core = 8
NJ = NH // 2        # head pairs per core = 4
P = 128
NQS = S // P        # 16 q subtiles
NKC = S // P        # 16 k chunks
KO = D // P         # 8 contraction chunks for projections
QC = 512            # projection s-chunk

F32 = mybir.dt.float32
BF16 = mybir.dt.bfloat16
EXP = mybir.ActivationFunctionType.Exp

_CACHE: dict = {}


def _emit(ctx: ExitStack, tc, xt, wq, wk, wv, wo, tri, ident, out):
    nc = tc.nc

    persist = ctx.enter_context(tc.tile_pool(name="persist", bufs=1))
    xt_sb = persist.tile([P, KO, S], BF16)
    wq_sb = persist.tile([P, KO, HD], BF16)
    wk_sb = persist.tile([P, KO, HD], BF16)
    wv_sb = persist.tile([P, KO, HD], BF16)
    wo_sb = persist.tile([P, NJ, D], BF16)
    qt_ev = persist.tile([P, NJ, S], BF16)
    qt_od = persist.tile([P, NJ, S], BF16)
    kt_sb = persist.tile([P, NJ, S], BF16)
    v_sb = persist.tile([P, NKC, NH, DK + 1], BF16)
    ctxT_sb = persist.tile([P, NJ, NQS, P], BF16)
    tri_sb = persist.tile([P, 1, P], BF16)
    id_sb = persist.tile([P, P], BF16)
    warm_sb = persist.tile([P, 2], BF16)

    xt_r = xt.rearrange("(o p) s -> p o s", p=P)
    wq_r = wq.rearrange("(o p) m -> p o m", p=P)
    wk_r = wk.rearrange("(o p) m -> p o m", p=P)
    wv_r = wv.rearrange("(o p) m -> p o m", p=P)
    wo_r = wo.rearrange("(j p) o -> p j o", p=P)

    with (
        tc.tile_pool(name="spps", bufs=2, space="PSUM") as spps,
        tc.tile_pool(name="cxps", bufs=1, space="PSUM") as cxps,
        tc.tile_pool(name="mixps", bufs=2, space="PSUM") as mixps,
        tc.tile_pool(name="ptp", bufs=3) as ptp,
        tc.tile_pool(name="cnp", bufs=2) as cnp,
        tc.tile_pool(name="rrp", bufs=2) as rrp,
        tc.tile_pool(name="osb", bufs=3) as osb,
    ):
        # ---- input DMAs, interleaved so the first K-proj group is fully fed
        # by ~2.5us.  The prologue needs only wk + the first s-chunk of x, so
        # x's sc0 columns load first, the remainder streams in behind.
        # queues: SP wk/consts/wq/wo; Pool + ACT split x chunks; Pool wv.
        for ko in range(4):
            nc.sync.dma_start(wk_sb[:, ko, :], wk_r[:, ko, :])
        for ko in range(0, KO, 2):
            nc.gpsimd.dma_start(xt_sb[:, ko, 0:QC], xt_r[:, ko, 0:QC])
            nc.scalar.dma_start(xt_sb[:, ko + 1, 0:QC], xt_r[:, ko + 1, 0:QC])
        for ko in range(4, KO):
            nc.gpsimd.dma_start(wk_sb[:, ko, :], wk_r[:, ko, :])
        nc.sync.dma_start(id_sb[:], ident)
        nc.sync.dma_start(tri_sb[:, 0, :], tri)
        # exp activation-table warm-up: off the critical path
        nc.scalar.activation(warm_sb[:], id_sb[:, 0:2], EXP)
        for ko in range(KO):
            nc.sync.dma_start(wq_sb[:, ko, :], wq_r[:, ko, :])
            eng = nc.gpsimd if ko % 2 == 0 else nc.scalar
            eng.dma_start(xt_sb[:, ko, QC:S], xt_r[:, ko, QC:S])
        for ko in range(KO):
            nc.gpsimd.dma_start(wv_sb[:, ko, :], wv_r[:, ko, :])
        nc.sync.dma_start(wo_sb[:], wo_r)
        # ones column of V_aug (softmax denominator accumulator)
        nc.gpsimd.memset(v_sb[:, :, :, DK : DK + 1], 1.0)
        # Q is stored twice with the other head's rows zeroed so score
        # matmuls contract the full 128 partitions from base partition 0
        # (operands at base partition 64 fail on hardware).  The dead halves
        # are zeroed once, per s-chunk, paced so attn(0) is not blocked.
        nc.vector.memset(qt_ev[DK:P, :, 0:QC], 0.0)
        nc.vector.memset(qt_od[0:DK, :, 0:QC], 0.0)

        # ---- building blocks ------------------------------------------
        def drain_k(pj, j, sc):
            nc.vector.tensor_copy(kt_sb[:, j, sc * QC : (sc + 1) * QC], pj[:])

        def drain_q(pj, j, sc):
            nc.vector.tensor_copy(
                qt_ev[0:DK, j, sc * QC : (sc + 1) * QC], pj[0:DK, :])
            nc.vector.tensor_copy(
                qt_od[DK:P, j, sc * QC : (sc + 1) * QC], pj[DK:P, :])

        def emit_kq_group(drain, w_sb, j, sc):
            pj = mixps.tile([P, QC], F32, tag="mix", name="pj")
            for ko in range(KO):
                nc.tensor.matmul(
                    pj[:],
                    w_sb[:, ko, j * P : (j + 1) * P],
                    xt_sb[:, ko, sc * QC : (sc + 1) * QC],
                    start=(ko == 0),
                    stop=(ko == KO - 1),
                )
            drain(pj, j, sc)

        def emit_v_group(kc):
            vp = mixps.tile([P, HD], F32, tag="mix", name="vp")
            for ko in range(KO):
                nc.tensor.matmul(
                    vp[:],
                    xt_sb[:, ko, kc * P : (kc + 1) * P],
                    wv_sb[:, ko, :],
                    start=(ko == 0),
                    stop=(ko == KO - 1),
                )
            nc.vector.tensor_copy(
                v_sb[:, kc, :, 0:DK], vp.rearrange("p (h e) -> p h e", h=NH)
            )

        def emit_oproj_half(qs, no):
            op = mixps.tile([P, QC], F32, tag="mix", name="op")
            for j in range(NJ):
                nc.tensor.matmul(
                    op[:],
                    ctxT_sb[:, j, qs, :],
                    wo_sb[:, j, no * QC : (no + 1) * QC],
                    start=(j == 0),
                    stop=(j == NJ - 1),
                )
            o2 = osb.tile([P, QC], BF16, tag="o")
            nc.vector.tensor_copy(o2[:], op[:])
            nc.sync.dma_start(
                out[qs * P : (qs + 1) * P, no * QC : (no + 1) * QC], o2[:]
            )

        def emit_oproj(qs):
            for no in range(2):
                emit_oproj_half(qs, no)

        # ---- prologue: first-chunk projections -------------------------
        for j in range(NJ):
            emit_kq_group(drain_k, wk_sb, j, 0)
        for j in range(NJ):
            emit_kq_group(drain_q, wq_sb, j, 0)
        emit_v_group(0)

        # PE filler emitted inside the attention loops: later chunks'
        # projections plus deferred early output projections, broken into
        # ~0.4us micro-steps (2 matmuls each) so one step per kc exactly
        # covers the PE deficit under the exp-bound (ACT 1.04us/kc) stretches.
        # Each (deadline, thunk) step must run before attn(deadline) starts.
        from collections import deque

        def kq_steps(drain_fn, w_sb, j, sc, dl):
            st = {}

            def mk(k0):
                def f():
                    if k0 == 0:
                        st["pj"] = mixps.tile([P, QC], F32, tag="mix", name="pj")
                    for ko in (k0, k0 + 1):
                        nc.tensor.matmul(
                            st["pj"][:],
                            w_sb[:, ko, j * P : (j + 1) * P],
                            xt_sb[:, ko, sc * QC : (sc + 1) * QC],
                            start=(ko == 0),
                            stop=(ko == KO - 1),
                        )
                return f

            def drain():
                drain_fn(st["pj"], j, sc)

            return [(dl, mk(k)) for k in (0, 2, 4, 6)] + [(dl, drain)]

        def v_steps(kc, dl):
            st = {}

            def mk(k0):
                def f():
                    if k0 == 0:
                        st["vp"] = mixps.tile([P, HD], F32, tag="mix", name="vp")
                    for ko in (k0, k0 + 1):
                        nc.tensor.matmul(
                            st["vp"][:],
                            xt_sb[:, ko, kc * P : (kc + 1) * P],
                            wv_sb[:, ko, :],
                            start=(ko == 0),
                            stop=(ko == KO - 1),
                        )
                return f

            def drain():
                nc.vector.tensor_copy(
                    v_sb[:, kc, :, 0:DK],
                    st["vp"].rearrange("p (h e) -> p h e", h=NH))

            return [(dl, mk(k)) for k in (0, 2, 4, 6)] + [(dl, drain)]

        def op_steps(oqs, no, dl):
            st = {}

            def mk(j0):
                def f():
                    if j0 == 0:
                        st["op"] = mixps.tile([P, QC], F32, tag="mix", name="op")
                    for j in (j0, j0 + 1):
                        nc.tensor.matmul(
                            st["op"][:],
                            ctxT_sb[:, j, oqs, :],
                            wo_sb[:, j, no * QC : (no + 1) * QC],
                            start=(j == 0),
                            stop=(j == NJ - 1),
                        )
                return f

            def drain():
                o2 = osb.tile([P, QC], BF16, tag="o", name="o2")
                nc.vector.tensor_copy(o2[:], st["op"][:])
                nc.sync.dma_start(
                    out[oqs * P : (oqs + 1) * P, no * QC : (no + 1) * QC],
                    o2[:])

            return [(dl, mk(0)), (dl, mk(2)), (dl, drain)]

        def qzero_step(sc, dl):
            def f():
                nc.vector.memset(qt_ev[DK:P, :, sc * QC : (sc + 1) * QC], 0.0)
                nc.vector.memset(qt_od[0:DK, :, sc * QC : (sc + 1) * QC], 0.0)
            return [(dl, f)]

        steps = deque()
        for sc in (1, 2, 3):
            lo = 4 * sc
            steps.extend(qzero_step(sc, lo - 2))
            for kc in range(lo - 3, lo):
                steps.extend(v_steps(kc, kc))
            for j in range(NJ):
                steps.extend(kq_steps(drain_k, wk_sb, j, sc, lo))
            for j in range(NJ):
                steps.extend(kq_steps(drain_q, wq_sb, j, sc, lo))
            steps.extend(v_steps(lo, lo))
        for kc in (13, 14, 15):
            steps.extend(v_steps(kc, kc))
        for oqs in (0, 1, 2, 3, 4, 6, 7, 8, 9):  # OP(5) held as tail filler
            steps.extend(op_steps(oqs, 0, 99))
            steps.extend(op_steps(oqs, 1, 99))

        tri_b = tri_sb.broadcast_to([P, NH, P])

        def emit_norm(qs, cx4):
            # normalize (q is the partition dim -> broadcast along free)
            rr = rrp.tile([P, 2, 4, 1], F32, tag="rr", name="rr")
            nc.vector.reciprocal(rr[:], cx4[:, :, :, DK : DK + 1])
            cn = cnp.tile([P, 2, 4, DK], BF16, tag="cn", name="cn")
            nc.vector.tensor_mul(
                cn[:], cx4[:, :, :, 0:DK], rr.broadcast_to([P, 2, 4, DK]))
            return cn

        def emit_transp(qs, cn):
            # transpose to pair layout: one PE transpose per head pair
            tp = mixps.tile([P, NJ, P], BF16, tag="mix", name="tp")
            for j in range(NJ):
                nc.tensor.matmul(
                    tp[:, j, :],
                    cn[:, (2 * j) // 4, (2 * j) % 4 : (2 * j) % 4 + 2, :],
                    id_sb[:],
                    is_transpose=True,
                    start=True,
                    stop=True,
                    skip_group_check=True,
                )
            nc.vector.tensor_copy(ctxT_sb[:, :, qs, :], tp[:])

        # ---- main q-tile sweep ------------------------------------------
        # The norm/transpose/oproj of q-tile qs is deferred into the first kc
        # steps of qs+1, so PE never waits on the DVE norm chain at a q-tile
        # boundary: the next tile's scores run under it.
        import os
        nqs_lim = int(os.environ.get("NQS_LIM", NQS))
        pending = None  # (qs, cx4) awaiting norm/transpose/output-projection
        for qs in range(nqs_lim):
            nkc = qs + 1
            pcn = None
            if pending is not None:
                pcn = emit_norm(pending[0], pending[1])  # DVE only
            # 4 heads per 512-f32 PSUM bank (65-wide groups must not cross a
            # bank boundary): head h lives at cx4[:, h//4, h%4, :]
            cx = cxps.tile([P, 2, 512], F32, tag="cx", name="cx")
            cx4 = cx[:, :, 0 : 4 * (DK + 1)].rearrange(
                "p b (h e) -> p b h e", h=4)
            nc.vector.memset(cx4[:], 0.0)
            # force any filler whose deadline has arrived
            while steps and steps[0][0] <= qs:
                steps.popleft()[1]()
            for kc in range(nkc):
                sp = spps.tile([P, NH, P], F32, tag="sp", name="sp")
                for h in range(NH):
                    j = h // 2
                    qsrc = qt_ev if h % 2 == 0 else qt_od
                    nc.tensor.matmul(
                        sp[:, h, :],
                        kt_sb[:, j, kc * P : (kc + 1) * P],
                        qsrc[:, j, qs * P : (qs + 1) * P],
                        start=True,
                        stop=True,
                        skip_group_check=True,
                    )
                pt = ptp.tile([P, NH, P], BF16, tag="pt", name="pt")
                nc.scalar.activation(pt[:], sp[:], EXP)
                if kc == qs:
                    nc.vector.tensor_mul(pt[:], pt[:], tri_b)
                if pending is not None:
                    # place the deferred transpose/oproj deep enough into this
                    # tile's kc steps that the DVE norm chain and the ctxT
                    # copy complete under preceding PE work
                    if kc == min(1, nkc - 1):
                        emit_transp(pending[0], pcn)
                    if pending[0] >= 10:
                        if kc == 2:
                            emit_oproj_half(pending[0], 0)
                        if kc == 3:
                            emit_oproj_half(pending[0], 1)
                # filler micro-steps sized to this kc step's PE deficit under
                # the exp rate: extra at the tile boundary (kc 0), none where
                # the inline oproj halves already fill (kc 2-3)
                if kc == 0:
                    want = 2
                elif kc in (2, 3) and pending is not None and pending[0] >= 10:
                    want = 0
                else:
                    want = 1
                for _ in range(want):
                    if steps:
                        steps.popleft()[1]()
                for h in range(NH):
                    nc.tensor.matmul(
                        cx4[:, h // 4, h % 4, :],
                        pt[:, h, :],
                        v_sb[:, kc, h, :],
                        start=False,
                        stop=(kc == nkc - 1),
                        skip_group_check=True,
                    )
            pending = (qs, cx4)
        # tail: last q-tile's norm/transpose/projection, with the held-back
        # OP(5) (plus any queue remainder) giving PE work while the DVE norm
        # chain and ctxT copy land
        if pending is not None and nqs_lim == NQS:
            cn15 = emit_norm(pending[0], pending[1])
            while steps:
                steps.popleft()[1]()
            emit_oproj_half(5, 0)
            emit_transp(pending[0], cn15)
            emit_oproj_half(5, 1)
            emit_oproj(pending[0])


def build_nc():
    nc = bacc.Bacc("TRN2", target_bir_lowering=False, debug=False)
    xt = nc.dram_tensor("xt", [D, S], BF16, kind="ExternalInput").ap()
    wq = nc.dram_tensor("wq", [D, HD], BF16, kind="ExternalInput").ap()
    wk = nc.dram_tensor("wk", [D, HD], BF16, kind="ExternalInput").ap()
    wv = nc.dram_tensor("wv", [D, HD], BF16, kind="ExternalInput").ap()
    wo = nc.dram_tensor("wo", [HD, D], BF16, kind="ExternalInput").ap()
    tri = nc.dram_tensor("tri", [P, P], BF16, kind="ExternalInput").ap()
    ident = nc.dram_tensor("ident", [P, P], BF16, kind="ExternalInput").ap()
    out = nc.dram_tensor("out", [S, D], BF16, kind="ExternalOutput").ap()
    with tile.TileContext(nc) as tc:
        with ExitStack() as ctx:
            with nc.allow_low_precision(reason="bf16 kernel by design"):
                _emit(ctx, tc, xt, wq, wk, wv, wo, tri, ident, out)
    nc.compile()
    return nc


def make_in_maps(x, W_q, W_k, W_v, W_o):
    import ml_dtypes

    BF = ml_dtypes.bfloat16
    x = np.asarray(x, dtype=np.float32)
    # fold the 1/sqrt(dk)=1/8 softmax scale into W_q (exact power of two)
    WqT = np.ascontiguousarray(np.asarray(W_q, np.float32).T * 0.125).astype(BF)
    WkT = np.ascontiguousarray(np.asarray(W_k, np.float32).T).astype(BF)
    WvT = np.ascontiguousarray(np.asarray(W_v, np.float32).T).astype(BF)
    WoT = np.ascontiguousarray(np.asarray(W_o, np.float32).T).astype(BF)
    # tri[k, q] = 1 where q >= k (within a diagonal 128x128 block)
    tri = np.triu(np.ones((P, P), np.float32)).astype(BF)
    ident = np.eye(P, dtype=np.float32).astype(BF)
    in_maps = []
    for c in range(2 * B):
        b, g = c // 2, c % 2
        in_maps.append({
            "xt": np.ascontiguousarray(x[b].T).astype(BF),
            "wq": np.ascontiguousarray(WqT[:, g * HD : (g + 1) * HD]),
            "wk": np.ascontiguousarray(WkT[:, g * HD : (g + 1) * HD]),
            "wv": np.ascontiguousarray(WvT[:, g * HD : (g + 1) * HD]),
            "wo": np.ascontiguousarray(WoT[g * HD : (g + 1) * HD, :]),
            "tri": tri,
            "ident": ident,
        })
    return in_maps


def get_runner():
    """Build (once) and cache a jitted 8-core executor for the bass program.

    Returns run(in_maps) -> list of per-core {name: np.ndarray} outputs.
    Mirrors concourse.bass2jax.run_bass_via_pjrt but caches the jitted
    callable so repeat kernel() calls skip re-lowering/compiling.
    """
    if "runner" in _CACHE:
        return _CACHE["runner"]
    import jax
    from jax.experimental.shard_map import shard_map
    from jax.sharding import Mesh, PartitionSpec
    from concourse import mybir as _mb
    from concourse.bass2jax import (
        _bass_exec_p, install_neuronx_cc_hook, partition_id_tensor)

    install_neuronx_cc_hook()
    nc = build_nc()
    n_cores = 2 * B

    partition_name = (nc.partition_id_tensor.name
                      if nc.partition_id_tensor else None)
    in_names, out_names, out_avals = [], [], []
    for alloc in nc.m.functions[0].allocations:
        if not isinstance(alloc, _mb.MemoryLocationSet):
            continue
        name = alloc.memorylocations[0].name
        if alloc.kind == "ExternalInput":
            if name != partition_name:
                in_names.append(name)
        elif alloc.kind == "ExternalOutput":
            out_names.append(name)
            out_avals.append(jax.core.ShapedArray(
                tuple(alloc.tensor_shape), _mb.dt.np(alloc.dtype)))
    n_params = len(in_names)
    all_names = in_names + out_names
    if partition_name is not None:
        all_names = all_names + [partition_name]

    def _body(*args):
        operands = list(args)
        if partition_name is not None:
            operands.append(partition_id_tensor())
        outs = _bass_exec_p.bind(
            *operands,
            out_avals=tuple(out_avals),
            in_names=tuple(all_names),
            out_names=tuple(out_names),
            lowering_input_output_aliases=(),
            sim_require_finite=True,
            sim_require_nnan=True,
            nc=nc,
        )
        return tuple(outs)

    devices = jax.devices()[:n_cores]
    mesh = Mesh(np.asarray(devices), ("core",))
    n_outs = len(out_names)
    sharded = jax.jit(
        shard_map(
            _body, mesh=mesh,
            in_specs=(PartitionSpec("core"),) * (n_params + n_outs),
            out_specs=(PartitionSpec("core"),) * n_outs,
            check_rep=False,
        ),
        donate_argnums=tuple(range(n_params, n_params + n_outs)),
        keep_unused=True,
    )

    def run(in_maps, device_arrays=None):
        concat_in = device_arrays if device_arrays is not None else [
            np.concatenate([np.asarray(in_maps[c][i_name])
                            for c in range(n_cores)], axis=0)
            for i_name in in_names
        ]
        concat_zeros = [
            np.zeros((n_cores * av.shape[0], *av.shape[1:]), av.dtype)
            for av in out_avals
        ]
        out_arrs = sharded(*concat_in, *concat_zeros)
        return [
            {name: np.asarray(out_arrs[i]).reshape(
                n_cores, *out_avals[i].shape)[c]
             for i, name in enumerate(out_names)}
            for c in range(n_cores)
        ]

    _CACHE["runner"] = (run, in_names, out_avals)
    return _CACHE["runner"]


def _run_cores(in_maps):
    """Execute the 8-core program; returns per-core {name: np.ndarray}."""
    from concourse._compat import axon_active
    if axon_active():
        # remote-accelerator proxy: use the cached jitted PJRT executor so
        # repeat calls skip re-lowering/compiling
        run, _, _ = get_runner()
        return run(in_maps)
    # native path (local /dev/neuron*): run_bass_kernel_spmd handles NEFF
    # compile caching + device execution directly
    if "nc" not in _CACHE:
        _CACHE["nc"] = build_nc()
    res = run_bass_kernel_spmd(_CACHE["nc"], in_maps, core_ids=list(range(2 * B)))
    _CACHE["last_exec_time_ns"] = res.exec_time_ns
    return res.results


def kernel(x, W_q, W_k, W_v, W_o):
    in_maps = make_in_maps(x, W_q, W_k, W_v, W_o)
    results = _run_cores(in_maps)
    out = np.empty((B, S, D), np.float32)
    for b in range(B):
        out[b] = (results[2 * b]["out"].astype(np.float32)
                  + results[2 * b + 1]["out"].astype(np.float32))
    return out


# revision 26
# speedup vs baseline: 1.2754x; 1.0348x over previous
"""Multi-head causal self-attention on 8 Trainium2 NeuronCores.

Problem: B=4, S=2048, D=1024, H=16 heads (dk=64), fp32 in/out, causal softmax.

Sharding: hybrid batch x head-group. Core c handles batch b = c//2 and head
group g = c%2 (8 heads = 512 dims). Each core computes QKV projections for
its head group, causal attention, and a partial output projection over its
512 context dims. The host sums the two bf16 partials per batch in fp32.

Device-side design (per core), all matmul operands bf16 (PSUM fp32):
  - Cost model charges matmuls by output free-size only, so every matmul is
    arranged to produce 128 output partitions per moving row where possible.
  - Q^T/K^T [128, pair, S]: partition block = head pair (64 rows each head).
  - Scores S^T[k, q] per (q-tile 128, k-chunk 128): 8 matmuls (contraction
    dk=64) into one 2-bank PSUM tile [128, 8head, 128q]; one exp (ACT) over
    all 8 heads; triangular mask multiply on the diagonal chunk only.
  - P@V runs TRANSPOSED: out ctx[q 128, 8, dv 64+1] with lhsT = exp(S^T)
    [k,q] and rhs = V_aug [k, 65] (ones column -> softmax denominator), so
    each PV matmul moves only 65 rows instead of 512.  The ctx accumulator
    PSUM tile is memset-zeroed once per q-tile and all PV matmuls use
    start=False (multiple interleaved accumulation groups share banks; the
    2KB zeroing granularity of start=True would clobber neighbours).
  - Normalization is a per-partition broadcast multiply on DVE (q is the
    partition dim), then one PE transpose per head PAIR ([q,2*64] -> pair
    layout [128, q]) readies ctx^T as the lhsT of the output projection.
  - Output projection accumulates 4 pairs x 512 cols in PSUM; bf16 partial
    written to DRAM; host adds the two head-group partials per batch.
  - Schedule: forward q-tile sweep with K/Q/V projection groups and
    deferred output projections interleaved into the attention loops as PE
    filler under the exp-bound (ACT) stretches.
"""

import numpy as np
from contextlib import ExitStack

import concourse.bass as bass
import concourse.tile as tile
from concourse import bacc, mybir
from concourse.bass_utils import run_bass_kernel_spmd

B, S, D = 4, 2048, 1024
H16 = 16
DK = 64
G = 2               # head groups (cores per batch)
HD = D // G         # per-core head dims = 512 (8 heads)
NH = HD // DK       # heads per core = 8
NJ = NH // 2        # head pairs per core = 4
P = 128
NQS = S // P        # 16 q subtiles
NKC = S // P        # 16 k chunks
KO = D // P         # 8 contraction chunks for projections
QC = 512            # projection s-chunk

F32 = mybir.dt.float32
BF16 = mybir.dt.bfloat16
EXP = mybir.ActivationFunctionType.Exp

_CACHE: dict = {}


def _emit(ctx: ExitStack, tc, xt, wq, wk, wv, wo, tri, ident, out):
    nc = tc.nc

    persist = ctx.enter_context(tc.tile_pool(name="persist", bufs=1))
    xt_sb = persist.tile([P, KO, S], BF16)
    wq_sb = persist.tile([P, KO, HD], BF16)
    wk_sb = persist.tile([P, KO, HD], BF16)
    wv_sb = persist.tile([P, KO, HD], BF16)
    wo_sb = persist.tile([P, NJ, D], BF16)
    qt_ev = persist.tile([P, NJ, S], BF16)
    qt_od = persist.tile([P, NJ, S], BF16)
    kt_sb = persist.tile([P, NJ, S], BF16)
    v_sb = persist.tile([P, NKC, NH, DK + 1], BF16)
    ctxT_sb = persist.tile([P, NJ, NQS, P], BF16)
    tri_sb = persist.tile([P, 1, P], BF16)
    id_sb = persist.tile([P, P], BF16)
    warm_sb = persist.tile([P, 2], BF16)

    xt_r = xt.rearrange("(o p) s -> p o s", p=P)
    wq_r = wq.rearrange("(o p) m -> p o m", p=P)
    wk_r = wk.rearrange("(o p) m -> p o m", p=P)
    wv_r = wv.rearrange("(o p) m -> p o m", p=P)
    wo_r = wo.rearrange("(j p) o -> p j o", p=P)

    with (
        tc.tile_pool(name="spps", bufs=2, space="PSUM") as spps,
        tc.tile_pool(name="cxps", bufs=1, space="PSUM") as cxps,
        tc.tile_pool(name="mixps", bufs=2, space="PSUM") as mixps,
        tc.tile_pool(name="ptp", bufs=3) as ptp,
        tc.tile_pool(name="cnp", bufs=2) as cnp,
        tc.tile_pool(name="rrp", bufs=2) as rrp,
        tc.tile_pool(name="osb", bufs=3) as osb,
    ):
        # ---- input DMAs, interleaved so the first K-proj group is fully fed
        # by ~2.5us.  The prologue needs only wk + the first s-chunk of x, so
        # x's sc0 columns load first, the remainder streams in behind.
        # queues: SP wk/consts/wq/wo; Pool + ACT split x chunks; Pool wv.
        for ko in range(4):
            nc.sync.dma_start(wk_sb[:, ko, :], wk_r[:, ko, :])
        for ko in range(0, KO, 2):
            nc.gpsimd.dma_start(xt_sb[:, ko, 0:QC], xt_r[:, ko, 0:QC])
            nc.scalar.dma_start(xt_sb[:, ko + 1, 0:QC], xt_r[:, ko + 1, 0:QC])
        for ko in range(4, KO):
            nc.gpsimd.dma_start(wk_sb[:, ko, :], wk_r[:, ko, :])
        nc.sync.dma_start(id_sb[:], ident)
        nc.sync.dma_start(tri_sb[:, 0, :], tri)
        # exp activation-table warm-up: off the critical path
        nc.scalar.activation(warm_sb[:], id_sb[:, 0:2], EXP)
        for ko in range(KO):
            nc.sync.dma_start(wq_sb[:, ko, :], wq_r[:, ko, :])
            eng = nc.gpsimd if ko % 2 == 0 else nc.scalar
            eng.dma_start(xt_sb[:, ko, QC:S], xt_r[:, ko, QC:S])
        for ko in range(KO):
            nc.gpsimd.dma_start(wv_sb[:, ko, :], wv_r[:, ko, :])
        nc.sync.dma_start(wo_sb[:], wo_r)
        # ones column of V_aug (softmax denominator accumulator)
        nc.gpsimd.memset(v_sb[:, :, :, DK : DK + 1], 1.0)
        # Q is stored twice with the other head's rows zeroed so score
        # matmuls contract the full 128 partitions from base partition 0
        # (operands at base partition 64 fail on hardware).  The dead halves
        # are zeroed once, per s-chunk, paced so attn(0) is not blocked.
        nc.gpsimd.memset(qt_ev[DK:P, :, 0:QC], 0.0)
        nc.gpsimd.memset(qt_od[0:DK, :, 0:QC], 0.0)

        # ---- building blocks ------------------------------------------
        def drain_k(pj, j, sc):
            nc.vector.tensor_copy(kt_sb[:, j, sc * QC : (sc + 1) * QC], pj[:])

        def drain_q(pj, j, sc):
            nc.vector.tensor_copy(
                qt_ev[0:DK, j, sc * QC : (sc + 1) * QC], pj[0:DK, :])
            nc.vector.tensor_copy(
                qt_od[DK:P, j, sc * QC : (sc + 1) * QC], pj[DK:P, :])

        def emit_kq_group(drain, w_sb, j, sc):
            pj = mixps.tile([P, QC], F32, tag="mix", name="pj")
            for ko in range(KO):
                nc.tensor.matmul(
                    pj[:],
                    w_sb[:, ko, j * P : (j + 1) * P],
                    xt_sb[:, ko, sc * QC : (sc + 1) * QC],
                    start=(ko == 0),
                    stop=(ko == KO - 1),
                )
            drain(pj, j, sc)

        def emit_v_group(kc):
            vp = mixps.tile([P, HD], F32, tag="mix", name="vp")
            for ko in range(KO):
                nc.tensor.matmul(
                    vp[:],
                    xt_sb[:, ko, kc * P : (kc + 1) * P],
                    wv_sb[:, ko, :],
                    start=(ko == 0),
                    stop=(ko == KO - 1),
                )
            nc.vector.tensor_copy(
                v_sb[:, kc, :, 0:DK], vp.rearrange("p (h e) -> p h e", h=NH)
            )

        def emit_oproj_half(qs, no):
            op = mixps.tile([P, QC], F32, tag="mix", name="op")
            for j in range(NJ):
                nc.tensor.matmul(
                    op[:],
                    ctxT_sb[:, j, qs, :],
                    wo_sb[:, j, no * QC : (no + 1) * QC],
                    start=(j == 0),
                    stop=(j == NJ - 1),
                )
            o2 = osb.tile([P, QC], BF16, tag="o")
            nc.vector.tensor_copy(o2[:], op[:])
            nc.sync.dma_start(
                out[qs * P : (qs + 1) * P, no * QC : (no + 1) * QC], o2[:]
            )

        def emit_oproj(qs):
            for no in range(2):
                emit_oproj_half(qs, no)

        # ---- prologue: first-chunk projections -------------------------
        for j in range(NJ):
            emit_kq_group(drain_k, wk_sb, j, 0)
        for j in range(NJ):
            emit_kq_group(drain_q, wq_sb, j, 0)
        emit_v_group(0)

        # PE filler emitted inside the attention loops: later chunks'
        # projections plus deferred early output projections, broken into
        # ~0.4us micro-steps (2 matmuls each) so one step per kc exactly
        # covers the PE deficit under the exp-bound (ACT 1.04us/kc) stretches.
        # Each (deadline, thunk) step must run before attn(deadline) starts.
        from collections import deque

        def kq_steps(drain_fn, w_sb, j, sc, dl):
            st = {}

            def mk(k0):
                def f():
                    if k0 == 0:
                        st["pj"] = mixps.tile([P, QC], F32, tag="mix", name="pj")
                    for ko in (k0, k0 + 1):
                        nc.tensor.matmul(
                            st["pj"][:],
                            w_sb[:, ko, j * P : (j + 1) * P],
                            xt_sb[:, ko, sc * QC : (sc + 1) * QC],
                            start=(ko == 0),
                            stop=(ko == KO - 1),
                        )
                return f

            def drain():
                drain_fn(st["pj"], j, sc)

            return [(dl, mk(k)) for k in (0, 2, 4, 6)] + [(dl, drain)]

        def v_steps(kc, dl):
            st = {}

            def mk(k0):
                def f():
                    if k0 == 0:
                        st["vp"] = mixps.tile([P, HD], F32, tag="mix", name="vp")
                    for ko in (k0, k0 + 1):
                        nc.tensor.matmul(
                            st["vp"][:],
                            xt_sb[:, ko, kc * P : (kc + 1) * P],
                            wv_sb[:, ko, :],
                            start=(ko == 0),
                            stop=(ko == KO - 1),
                        )
                return f

            def drain():
                nc.vector.tensor_copy(
                    v_sb[:, kc, :, 0:DK],
                    st["vp"].rearrange("p (h e) -> p h e", h=NH))

            return [(dl, mk(k)) for k in (0, 2, 4, 6)] + [(dl, drain)]

        def op_steps(oqs, no, dl):
            st = {}

            def mk(j0):
                def f():
                    if j0 == 0:
                        st["op"] = mixps.tile([P, QC], F32, tag="mix", name="op")
                    for j in (j0, j0 + 1):
                        nc.tensor.matmul(
                            st["op"][:],
                            ctxT_sb[:, j, oqs, :],
                            wo_sb[:, j, no * QC : (no + 1) * QC],
                            start=(j == 0),
                            stop=(j == NJ - 1),
                        )
                return f

            def drain():
                o2 = osb.tile([P, QC], BF16, tag="o", name="o2")
                nc.vector.tensor_copy(o2[:], st["op"][:])
                nc.sync.dma_start(
                    out[oqs * P : (oqs + 1) * P, no * QC : (no + 1) * QC],
                    o2[:])

            return [(dl, mk(0)), (dl, mk(2)), (dl, drain)]

        def qzero_step(sc, dl):
            def f():
                nc.gpsimd.memset(qt_ev[DK:P, :, sc * QC : (sc + 1) * QC], 0.0)
                nc.gpsimd.memset(qt_od[0:DK, :, sc * QC : (sc + 1) * QC], 0.0)
            return [(dl, f)]

        steps = deque()
        for sc in (1, 2, 3):
            lo = 4 * sc
            steps.extend(qzero_step(sc, lo - 2))
            for kc in range(lo - 3, lo):
                steps.extend(v_steps(kc, kc))
            for j in range(NJ):
                steps.extend(kq_steps(drain_k, wk_sb, j, sc, lo))
            for j in range(NJ):
                steps.extend(kq_steps(drain_q, wq_sb, j, sc, lo))
            steps.extend(v_steps(lo, lo))
        for kc in (13, 14, 15):
            steps.extend(v_steps(kc, kc))
        for oqs in (0, 1, 2, 3, 4, 6, 7, 8, 9):  # OP(5) held as tail filler
            steps.extend(op_steps(oqs, 0, 99))
            steps.extend(op_steps(oqs, 1, 99))

        tri_b = tri_sb.broadcast_to([P, NH, P])

        def emit_norm(qs, cx4):
            # normalize (q is the partition dim -> broadcast along free)
            rr = rrp.tile([P, 2, 4, 1], F32, tag="rr", name="rr")
            nc.vector.reciprocal(rr[:], cx4[:, :, :, DK : DK + 1])
            cn = cnp.tile([P, 2, 4, DK], BF16, tag="cn", name="cn")
            nc.vector.tensor_mul(
                cn[:], cx4[:, :, :, 0:DK], rr.broadcast_to([P, 2, 4, DK]))
            return cn

        def emit_transp(qs, cn):
            # transpose to pair layout: one PE transpose per head pair
            tp = mixps.tile([P, NJ, P], BF16, tag="mix", name="tp")
            for j in range(NJ):
                nc.tensor.matmul(
                    tp[:, j, :],
                    cn[:, (2 * j) // 4, (2 * j) % 4 : (2 * j) % 4 + 2, :],
                    id_sb[:],
                    is_transpose=True,
                    start=True,
                    stop=True,
                    skip_group_check=True,
                )
            nc.vector.tensor_copy(ctxT_sb[:, :, qs, :], tp[:])

        # ---- main q-tile sweep ------------------------------------------
        # The norm/transpose/oproj of q-tile qs is deferred into the first kc
        # steps of qs+1, so PE never waits on the DVE norm chain at a q-tile
        # boundary: the next tile's scores run under it.
        import os
        nqs_lim = int(os.environ.get("NQS_LIM", NQS))
        pending = None  # (qs, cx4) awaiting norm/transpose/output-projection
        for qs in range(nqs_lim):
            nkc = qs + 1
            pcn = None
            if pending is not None:
                pcn = emit_norm(pending[0], pending[1])  # DVE only
            # 4 heads per 512-f32 PSUM bank (65-wide groups must not cross a
            # bank boundary): head h lives at cx4[:, h//4, h%4, :]
            cx = cxps.tile([P, 2, 512], F32, tag="cx", name="cx")
            cx4 = cx[:, :, 0 : 4 * (DK + 1)].rearrange(
                "p b (h e) -> p b h e", h=4)
            nc.vector.memset(cx4[:], 0.0)
            # force any filler whose deadline has arrived
            while steps and steps[0][0] <= qs:
                steps.popleft()[1]()
            for kc in range(nkc):
                sp = spps.tile([P, NH, P], F32, tag="sp", name="sp")
                for h in range(NH):
                    j = h // 2
                    qsrc = qt_ev if h % 2 == 0 else qt_od
                    nc.tensor.matmul(
                        sp[:, h, :],
                        kt_sb[:, j, kc * P : (kc + 1) * P],
                        qsrc[:, j, qs * P : (qs + 1) * P],
                        start=True,
                        stop=True,
                        skip_group_check=True,
                    )
                pt = ptp.tile([P, NH, P], BF16, tag="pt", name="pt")
                nc.scalar.activation(pt[:], sp[:], EXP)
                if kc == qs:
                    nc.vector.tensor_mul(pt[:], pt[:], tri_b)
                if pending is not None:
                    # place the deferred transpose/oproj deep enough into this
                    # tile's kc steps that the DVE norm chain and the ctxT
                    # copy complete under preceding PE work
                    if kc == min(1, nkc - 1):
                        emit_transp(pending[0], pcn)
                    if pending[0] >= 10:
                        if kc == 2:
                            emit_oproj_half(pending[0], 0)
                        if kc == 3:
                            emit_oproj_half(pending[0], 1)
                # filler micro-steps sized to this kc step's PE deficit under
                # the exp rate: extra at the tile boundary (kc 0), none where
                # the inline oproj halves already fill (kc 2-3)
                if kc == 0:
                    want = 2
                elif kc in (2, 3) and pending is not None and pending[0] >= 10:
                    want = 0
                else:
                    want = 1
                for _ in range(want):
                    if steps:
                        steps.popleft()[1]()
                for h in range(NH):
                    nc.tensor.matmul(
                        cx4[:, h // 4, h % 4, :],
                        pt[:, h, :],
                        v_sb[:, kc, h, :],
                        start=False,
                        stop=(kc == nkc - 1),
                        skip_group_check=True,
                    )
            pending = (qs, cx4)
        # tail: last q-tile's norm/transpose/projection, with the held-back
        # OP(5) (plus any queue remainder) giving PE work while the DVE norm
        # chain and ctxT copy land
        if pending is not None and nqs_lim == NQS:
            cn15 = emit_norm(pending[0], pending[1])
            while steps:
                steps.popleft()[1]()
            emit_oproj_half(5, 0)
            emit_transp(pending[0], cn15)
            emit_oproj_half(5, 1)
            emit_oproj(pending[0])


def build_nc():
    nc = bacc.Bacc("TRN2", target_bir_lowering=False, debug=False)
    xt = nc.dram_tensor("xt", [D, S], BF16, kind="ExternalInput").ap()
    wq = nc.dram_tensor("wq", [D, HD], BF16, kind="ExternalInput").ap()
    wk = nc.dram_tensor("wk", [D, HD], BF16, kind="ExternalInput").ap()
    wv = nc.dram_tensor("wv", [D, HD], BF16, kind="ExternalInput").ap()
    wo = nc.dram_tensor("wo", [HD, D], BF16, kind="ExternalInput").ap()
    tri = nc.dram_tensor("tri", [P, P], BF16, kind="ExternalInput").ap()
    ident = nc.dram_tensor("ident", [P, P], BF16, kind="ExternalInput").ap()
    out = nc.dram_tensor("out", [S, D], BF16, kind="ExternalOutput").ap()
    with tile.TileContext(nc) as tc:
        with ExitStack() as ctx:
            with nc.allow_low_precision(reason="bf16 kernel by design"):
                _emit(ctx, tc, xt, wq, wk, wv, wo, tri, ident, out)
    nc.compile()
    return nc


def make_in_maps(x, W_q, W_k, W_v, W_o):
    import ml_dtypes

    BF = ml_dtypes.bfloat16
    x = np.asarray(x, dtype=np.float32)
    # fold the 1/sqrt(dk)=1/8 softmax scale into W_q (exact power of two)
    WqT = np.ascontiguousarray(np.asarray(W_q, np.float32).T * 0.125).astype(BF)
    WkT = np.ascontiguousarray(np.asarray(W_k, np.float32).T).astype(BF)
    WvT = np.ascontiguousarray(np.asarray(W_v, np.float32).T).astype(BF)
    WoT = np.ascontiguousarray(np.asarray(W_o, np.float32).T).astype(BF)
    # tri[k, q] = 1 where q >= k (within a diagonal 128x128 block)
    tri = np.triu(np.ones((P, P), np.float32)).astype(BF)
    ident = np.eye(P, dtype=np.float32).astype(BF)
    in_maps = []
    for c in range(2 * B):
        b, g = c // 2, c % 2
        in_maps.append({
            "xt": np.ascontiguousarray(x[b].T).astype(BF),
            "wq": np.ascontiguousarray(WqT[:, g * HD : (g + 1) * HD]),
            "wk": np.ascontiguousarray(WkT[:, g * HD : (g + 1) * HD]),
            "wv": np.ascontiguousarray(WvT[:, g * HD : (g + 1) * HD]),
            "wo": np.ascontiguousarray(WoT[g * HD : (g + 1) * HD, :]),
            "tri": tri,
            "ident": ident,
        })
    return in_maps


def get_runner():
    """Build (once) and cache a jitted 8-core executor for the bass program.

    Returns run(in_maps) -> list of per-core {name: np.ndarray} outputs.
    Mirrors concourse.bass2jax.run_bass_via_pjrt but caches the jitted
    callable so repeat kernel() calls skip re-lowering/compiling.
    """
    if "runner" in _CACHE:
        return _CACHE["runner"]
    import jax
    from jax.experimental.shard_map import shard_map
    from jax.sharding import Mesh, PartitionSpec
    from concourse import mybir as _mb
    from concourse.bass2jax import (
        _bass_exec_p, install_neuronx_cc_hook, partition_id_tensor)

    install_neuronx_cc_hook()
    nc = build_nc()
    n_cores = 2 * B

    partition_name = (nc.partition_id_tensor.name
                      if nc.partition_id_tensor else None)
    in_names, out_names, out_avals = [], [], []
    for alloc in nc.m.functions[0].allocations:
        if not isinstance(alloc, _mb.MemoryLocationSet):
            continue
        name = alloc.memorylocations[0].name
        if alloc.kind == "ExternalInput":
            if name != partition_name:
                in_names.append(name)
        elif alloc.kind == "ExternalOutput":
            out_names.append(name)
            out_avals.append(jax.core.ShapedArray(
                tuple(alloc.tensor_shape), _mb.dt.np(alloc.dtype)))
    n_params = len(in_names)
    all_names = in_names + out_names
    if partition_name is not None:
        all_names = all_names + [partition_name]

    def _body(*args):
        operands = list(args)
        if partition_name is not None:
            operands.append(partition_id_tensor())
        outs = _bass_exec_p.bind(
            *operands,
            out_avals=tuple(out_avals),
            in_names=tuple(all_names),
            out_names=tuple(out_names),
            lowering_input_output_aliases=(),
            sim_require_finite=True,
            sim_require_nnan=True,
            nc=nc,
        )
        return tuple(outs)

    devices = jax.devices()[:n_cores]
    mesh = Mesh(np.asarray(devices), ("core",))
    n_outs = len(out_names)
    sharded = jax.jit(
        shard_map(
            _body, mesh=mesh,
            in_specs=(PartitionSpec("core"),) * (n_params + n_outs),
            out_specs=(PartitionSpec("core"),) * n_outs,
            check_rep=False,
        ),
        donate_argnums=tuple(range(n_params, n_params + n_outs)),
        keep_unused=True,
    )

    def run(in_maps, device_arrays=None):
        concat_in = device_arrays if device_arrays is not None else [
            np.concatenate([np.asarray(in_maps[c][i_name])
                            for c in range(n_cores)], axis=0)
            for i_name in in_names
        ]
        concat_zeros = [
            np.zeros((n_cores * av.shape[0], *av.shape[1:]), av.dtype)
            for av in out_avals
        ]
        out_arrs = sharded(*concat_in, *concat_zeros)
        return [
            {name: np.asarray(out_arrs[i]).reshape(
                n_cores, *out_avals[i].shape)[c]
             for i, name in enumerate(out_names)}
            for c in range(n_cores)
        ]

    _CACHE["runner"] = (run, in_names, out_avals)
    return _CACHE["runner"]


def _run_cores(in_maps):
    """Execute the 8-core program; returns per-core {name: np.ndarray}."""
    from concourse._compat import axon_active
    if axon_active():
        # remote-accelerator proxy: use the cached jitted PJRT executor so
        # repeat calls skip re-lowering/compiling
        run, _, _ = get_runner()
        return run(in_maps)
    # native path (local /dev/neuron*): run_bass_kernel_spmd handles NEFF
    # compile caching + device execution directly
    if "nc" not in _CACHE:
        _CACHE["nc"] = build_nc()
    res = run_bass_kernel_spmd(_CACHE["nc"], in_maps, core_ids=list(range(2 * B)))
    _CACHE["last_exec_time_ns"] = res.exec_time_ns
    return res.results


def kernel(x, W_q, W_k, W_v, W_o):
    in_maps = make_in_maps(x, W_q, W_k, W_v, W_o)
    results = _run_cores(in_maps)
    out = np.empty((B, S, D), np.float32)
    for b in range(B):
        out[b] = (results[2 * b]["out"].astype(np.float32)
                  + results[2 * b + 1]["out"].astype(np.float32))
    return out
